# revision 24
# baseline (speedup 1.0000x reference)
"""Trainium2 Bass kernel for nn_AttentionFlow (trilinear attention flow layer).

Full inputs -> shard batch over 8 NeuronCores (2 batches/core) -> gather.

Per batch (C [1024,768], Q [128,768]):
  S[i,j] = w1.C_i + w2.Q_j + (C_i*w3).Q_j
  C2Q = softmax_j(masked S); A = C2Q @ Q
  Q2C = softmax_i(c-masked rowmax of S); Bctx = Q2C @ C
  out = [C | A | C*A | C*Bctx]

v4 design:
  - float32r everywhere the PE streams >=256 even columns (1 cyc/row vs 4
    for fp32; FP22 read truncation, ~1e-4).  Walrus requires fp32r matmul
    operands to be *produced* as fp32r, so C/qzb/qw3/l2/r2/qcorr/cmR are
    declared float32r end-to-end (bytes are plain fp32; non-PE readers
    bitcast back).
  - Scores computed transposed (S^T [m,n], J=512 streams) with q_logit and
    c_logit folded in by one K=2 matmul of host-built rows; PE-transposed
    back per 128-tile for the row-space softmax.
  - exp shifted by raw row max; masked-j E columns are annihilated by
    qzb = [Q rows zeroed | qbin col | pad], which also emits the softmax
    normalizer Z as an output column; c-masked rows fixed to reference
    semantics by a rank-1 cm x [sum_masked Q | n_masked | 0] into the psum.
  - Scheduling: per-iteration emission is phase-major (loads | C stores |
    transposes b0,b1 | scores b0,b1 | softmax/A b0,b1 | Q2C+CB b0,b1) so
    each engine FIFO streams without head-of-line stalls; C^T transposes
    stage 3-at-a-time into [128,384] psum tiles (2 copies per n-tile);
    the softmax phase emits block-transposes two tiles ahead; loads ride
    the otherwise-idle GPSIMD (SWDGE) queue so next-iteration loads issue
    early; stores ride SP, elementwise sits on DVE/ACT only.
"""

from contextlib import ExitStack

import numpy as np

import concourse.bass as bass
import concourse.tile as tile
from concourse import bacc, mybir
from concourse.bass_utils import run_bass_kernel_spmd
from concourse.masks import make_identity

F32 = mybir.dt.float32
F32R = mybir.dt.float32r
AX = mybir.AluOpType
ACTF = mybir.ActivationFunctionType

NEG = np.float32(-1e9)
NCORES = 8
NB = 2           # batches per core
N = 1024         # context length
M = 128          # query length
D = 768          # feature dim
NT = N // 128    # n-tiles per batch
KC = D // 128    # contraction chunks

_CACHE: dict = {}


def _f(ap):
    return ap.bitcast(F32)


def _r(ap):
    return ap.bitcast(F32R)


def _build_program(iters: int = 1, do_load: bool = True,
                   do_store: bool = True) -> bass.Bass:
    nc = bacc.Bacc("TRN2", target_bir_lowering=False, debug=False)
    C_d = nc.declare_dram_parameter("C", [NB, N, D], F32R, isOutput=False)
    qzb_d = nc.declare_dram_parameter("qzb", [NB, M, D + 2], F32R, isOutput=False)
    qw3_d = nc.declare_dram_parameter("qw3", [NB, KC, 128, M], F32R, isOutput=False)
    l2_d = nc.declare_dram_parameter("l2", [NB, 2, M], F32R, isOutput=False)
    r2_d = nc.declare_dram_parameter("r2", [NB, 2, N], F32R, isOutput=False)
    qcorr_d = nc.declare_dram_parameter("qcorr", [NB, 1, D + 2], F32R, isOutput=False)
    cmT_d = nc.declare_dram_parameter("cmT", [NB, 128, NT], F32, isOutput=False)
    qmN_d = nc.declare_dram_parameter("qmN", [NB, 1, M], F32, isOutput=False)
    cmR_d = nc.declare_dram_parameter("cmR", [NB, 1, N], F32R, isOutput=False)
    out_d = nc.declare_dram_parameter("out", [NB, N, 4 * D], F32, isOutput=True)

    with ExitStack() as ctx:
        tc = ctx.enter_context(tile.TileContext(nc))
        consts = ctx.enter_context(tc.tile_pool(name="consts", bufs=1))
        cpool = ctx.enter_context(tc.tile_pool(name="cpool", bufs=2))
        ctpool = ctx.enter_context(tc.tile_pool(name="ctpool", bufs=2))
        qpool = ctx.enter_context(tc.tile_pool(name="qpool", bufs=2))
        stpool = ctx.enter_context(tc.tile_pool(name="stpool", bufs=4))
        epool = ctx.enter_context(tc.tile_pool(name="epool", bufs=4))
        spool = ctx.enter_context(tc.tile_pool(name="spool", bufs=6))
        bpool = ctx.enter_context(tc.tile_pool(name="bpool", bufs=2))
        stA = ctx.enter_context(tc.tile_pool(name="stA", bufs=3))
        stB = ctx.enter_context(tc.tile_pool(name="stB", bufs=3))
        ps_a = ctx.enter_context(tc.tile_pool(name="ps_a", bufs=2,
                                              space="PSUM"))
        ps_work = ctx.enter_context(tc.tile_pool(name="ps_work", bufs=1,
                                                 space="PSUM"))
        ps_bm = ctx.enter_context(tc.tile_pool(name="ps_bm", bufs=1,
                                               space="PSUM"))
        ps_bk = ctx.enter_context(tc.tile_pool(name="ps_bk", bufs=2,
                                               space="PSUM"))
        

        ident = consts.tile([128, 128], F32)
        make_identity(nc, ident)
        ident_r = consts.tile([128, 128], F32R)
        nc.vector.tensor_copy(out=ident_r, in_=ident)
        ones_row = consts.tile([1, 128], F32)
        nc.vector.memset(ones_row, 1.0)
        ones_row_r = consts.tile([1, 128], F32R)
        nc.vector.tensor_copy(out=ones_row_r, in_=ones_row)
        ones_col = consts.tile([128, 1], F32)
        nc.vector.memset(ones_col, 1.0)

        loop_ctx = tc.For_i(0, iters, 1) if iters > 1 else None
        if loop_ctx is not None:
            ctx.enter_context(loop_ctx)

        B = range(NB)
        c_big, qzb, qw3, l2, r2, qcorr, cmT, cmR = ({} for _ in range(8))
        qmN = {}
        s0c, negs0c, cmNc = {}, {}, {}
        ctall, st_sb, G = {}, {}, {}

        # ---------------- phase L: all loads (GPSIMD SWDGE queue) --------
        for b in B:
            c_big[b] = cpool.tile([128, NT, D], F32R, tag="c", name=f"cbig{b}")
            if do_load:
                for t in range(NT):
                    nc.gpsimd.dma_start(
                        out=c_big[b][:, t, :],
                        in_=C_d[b, t * 128:(t + 1) * 128, :])
            else:
                nc.vector.memset(_f(c_big[b]), 0.001 * (b + 1))
            qzb[b] = qpool.tile([128, D + 2], F32R, tag="qzb", name=f"qzb{b}")
            qw3[b] = qpool.tile([128, KC, M], F32R, tag="qw3", name=f"qw3{b}")
            l2[b] = bpool.tile([2, M], F32R, tag="l2", name=f"l2{b}")
            r2[b] = bpool.tile([2, N], F32R, tag="r2", name=f"r2{b}")
            qcorr[b] = bpool.tile([1, D + 2], F32R, tag="qcorr", name=f"qcorr{b}")
            cmT[b] = bpool.tile([128, NT], F32, tag="cmT", name=f"cmT{b}")
            cmR[b] = bpool.tile([1, N], F32R, tag="cmR", name=f"cmR{b}")
            qmN[b] = bpool.tile([1, M], F32, tag="qmN", name=f"qmN{b}")
            if do_load:
                nc.scalar.dma_start(out=qzb[b], in_=qzb_d[b])
                nc.scalar.dma_start(out=qw3[b],
                                    in_=qw3_d[b].rearrange("c p m -> p c m"))
                nc.scalar.dma_start(out=l2[b], in_=l2_d[b])
                nc.scalar.dma_start(out=r2[b], in_=r2_d[b])
                nc.scalar.dma_start(out=qcorr[b], in_=qcorr_d[b])
                nc.scalar.dma_start(out=cmT[b], in_=cmT_d[b])
                nc.scalar.dma_start(out=cmR[b], in_=cmR_d[b])
                nc.scalar.dma_start(out=qmN[b], in_=qmN_d[b])
            else:
                nc.vector.memset(_f(qzb[b]), 1.0)
                nc.vector.memset(_f(qw3[b]), 0.01)
                nc.vector.memset(_f(l2[b]), 0.5)
                nc.vector.memset(_f(r2[b]), 0.5)
                nc.vector.memset(_f(qcorr[b]), 0.5)
                nc.vector.memset(cmT[b], 0.0)
                nc.vector.memset(_f(cmR[b]), 0.0)
                nc.vector.memset(qmN[b], 0.0)

        # ------------- mask derivations + early C stores (SP) -----------
        for b in B:
            s0c[b] = bpool.tile([128, NT], F32, tag="s0c", name=f"s0c{b}")
            nc.vector.tensor_scalar(out=s0c[b], in0=cmT[b], scalar1=-1.0,
                                    scalar2=1.0, op0=AX.mult, op1=AX.add)
            negs0c[b] = bpool.tile([128, NT], F32, tag="negs0c", name=f"negs0c{b}")
            nc.vector.tensor_scalar_add(out=negs0c[b], in0=cmT[b], scalar1=-1.0)
            cmNc[b] = bpool.tile([128, NT], F32, tag="cmNc", name=f"cmNc{b}")
            nc.vector.tensor_scalar_mul(out=cmNc[b], in0=cmT[b],
                                        scalar1=float(NEG))
            if do_store:
                for t in range(NT):
                    nc.sync.dma_start(
                        out=out_d[b, t * 128:(t + 1) * 128, 0:D],
                        in_=_f(c_big[b][:, t, :]))

        # ------------- phase T: C^T, staged 3 chunks per psum tile -------
        for b in B:
            ctall[b] = ctpool.tile([128, NT, D], F32R, tag="ct", name=f"ctall{b}")
            for t in range(NT):
                for g in range(2):
                    tr_ps = ps_work.tile([128, 3 * 128], F32, tag="work")
                    for k in range(3):
                        c = 3 * g + k
                        nc.tensor.transpose(
                            _r(tr_ps[:, k * 128:(k + 1) * 128]),
                            c_big[b][:, t, c * 128:(c + 1) * 128], ident_r)
                    dst = ctall[b][:, t, g * 384:(g + 1) * 384]
                    if (t + g) % 2 == 0:
                        nc.vector.tensor_copy(out=dst, in_=_r(tr_ps))
                    else:
                        nc.scalar.copy(out=dst, in_=_r(tr_ps))

        # ---------------- phase S: scores S^T in two 512-halves ----------
        for b in B:
            st_sb[b] = [stpool.tile([128, 512], F32, tag="stsb",
                                    name=f"st{b}_{h}") for h in range(2)]
            for h in range(2):
                s_ps = ps_work.tile([128, 512], F32, tag="work")
                for c in range(KC):
                    nc.tensor.matmul(
                        s_ps, lhsT=qw3[b][:, c, :],
                        rhs=ctall[b][:, 4 * h:4 * h + 4, c * 128:(c + 1) * 128],
                        start=(c == 0), stop=False)
                nc.tensor.matmul(s_ps, lhsT=l2[b],
                                 rhs=r2[b][:, h * 512:(h + 1) * 512],
                                 start=False, stop=True)
                if h == 0:
                    nc.scalar.copy(out=st_sb[b][h], in_=s_ps)
                else:
                    nc.vector.tensor_copy(out=st_sb[b][h], in_=s_ps)

        # -------- phase A: per n-tile softmax row space + A + stores -----
        for b in B:
            G[b] = bpool.tile([128, NT], F32, tag="G", name=f"G{b}")
            blk = {}

            def emit_blk(t, b=b, blk=blk):
                blk[t] = ps_bk.tile([128, 128], F32, tag="bk", name=f"blk{b}_{t}")
                src = st_sb[b][t // 4][:, (t % 4) * 128:(t % 4 + 1) * 128]
                nc.tensor.transpose(blk[t], src, ident)

            blkm = {}

            def emit_blkm(t, b=b, blkm=blkm):
                blkm[t] = ps_bm.tile([128, 128], F32, tag="bm",
                                     name=f"blkm{b}_{t}")
                src = st_sb[b][t // 4][:, (t % 4) * 128:(t % 4 + 1) * 128]
                nc.tensor.matmul(blkm[t], lhsT=src, rhs=ident,
                                 is_transpose=True, start=True, stop=False)
                nc.tensor.matmul(blkm[t], lhsT=ones_row, rhs=qmN[b],
                                 start=False, stop=True)

            emit_blk(0)
            emit_blkm(0)
            emit_blk(1)
            emit_blkm(1)
            for t in range(NT):
                blk_ps = blk.pop(t)
                blkm_ps = blkm.pop(t)
                nrawmax = spool.tile([128, 1], F32, tag="nrawmax")
                nc.vector.reduce_max(out=nrawmax, in_=blk_ps,
                                     axis=mybir.AxisListType.X, negate=True)
                nc.vector.tensor_scalar(out=G[b][:, t:t + 1], in0=nrawmax,
                                        scalar1=negs0c[b][:, t:t + 1],
                                        scalar2=cmNc[b][:, t:t + 1],
                                        op0=AX.mult, op1=AX.add)
                nmmax = spool.tile([128, 1], F32, tag="nmmax")
                nc.vector.reduce_max(out=nmmax, in_=blkm_ps,
                                     axis=mybir.AxisListType.X, negate=True)
                biasT = spool.tile([128, 1], F32, tag="biasT")
                nc.vector.tensor_scalar_mul(out=biasT, in0=nmmax,
                                            scalar1=s0c[b][:, t:t + 1])
                E = epool.tile([128, M], F32R, tag="E")
                nc.scalar.activation(out=E, in_=blkm_ps, func=ACTF.Exp,
                                     bias=biasT, scale=s0c[b][:, t:t + 1])
                et_ps = ps_bk.tile([128, M], F32, tag="bk")
                nc.tensor.transpose(_r(et_ps), E, ident_r)
                et = epool.tile([128, M], F32R, tag="et")
                nc.vector.tensor_copy(out=et, in_=et_ps)
                if t + 2 < NT:
                    emit_blk(t + 2)
                    emit_blkm(t + 2)

                a_ps = ps_a.tile([128, D + 2], F32, tag="aps")
                aA = a_ps[:, 0:512]
                aB = a_ps[:, 512:D + 2]
                cmr_t = cmR[b][:, t * 128:(t + 1) * 128]
                nc.tensor.matmul(aA, lhsT=et, rhs=qzb[b][:, 0:512],
                                 start=True, stop=False)
                nc.tensor.matmul(aA, lhsT=cmr_t, rhs=qcorr[b][:, 0:512],
                                 start=False, stop=True)
                nc.tensor.matmul(aB, lhsT=et, rhs=qzb[b][:, 512:D + 2],
                                 start=True, stop=False)
                nc.tensor.matmul(aB, lhsT=cmr_t, rhs=qcorr[b][:, 512:D + 2],
                                 start=False, stop=True)
                zr = spool.tile([128, 1], F32, tag="zr")
                nc.vector.reciprocal(out=zr, in_=a_ps[:, D:D + 1])

                stage = stA.tile([128, 2 * D], F32, tag="stA")
                if t % 2 == 0:
                    nc.scalar.activation(out=stage[:, 0:512], in_=aA,
                                         func=ACTF.Copy, scale=zr)
                    nc.scalar.activation(out=stage[:, 512:D], in_=a_ps[:, 512:D],
                                         func=ACTF.Copy, scale=zr)
                else:
                    nc.vector.tensor_scalar_mul(out=stage[:, 0:512], in0=aA,
                                                scalar1=zr)
                    nc.vector.tensor_scalar_mul(out=stage[:, 512:D],
                                                in0=a_ps[:, 512:D], scalar1=zr)
                ca_eng = nc.gpsimd if t % 2 == 0 else nc.vector
                ca_eng.tensor_mul(out=stage[:, D:2 * D], in0=stage[:, 0:D],
                                  in1=_f(c_big[b][:, t, :]))
                if do_store:
                    nc.sync.dma_start(
                        out=out_d[b, t * 128:(t + 1) * 128, D:3 * D],
                        in_=stage)

        # ---------------- phase Q: Q2C global softmax + Bctx + CB --------
        for b in B:
            gt_ps = ps_bk.tile([NT, 128], F32, tag="bk")
            nc.tensor.transpose(gt_ps, G[b], ident)
            m8n = bpool.tile([NT, 1], F32, tag="m8n")
            nc.vector.reduce_max(out=m8n, in_=gt_ps, axis=mybir.AxisListType.X,
                                 negate=True)
            m8t_ps = ps_bk.tile([1, NT], F32, tag="bk")
            nc.tensor.transpose(m8t_ps, m8n, ident[0:NT, 0:NT])
            negMg = bpool.tile([1, 1], F32, tag="negMg")
            nc.vector.tensor_reduce(out=negMg, in_=m8t_ps,
                                    axis=mybir.AxisListType.X, op=AX.min)
            nm8_ps = ps_bk.tile([NT, 1], F32, tag="bk")
            nc.tensor.matmul(nm8_ps, lhsT=ones_row[:, 0:NT], rhs=negMg,
                             start=True, stop=True)
            nm8 = bpool.tile([NT, 1], F32, tag="nm8")
            nc.vector.tensor_copy(out=nm8, in_=nm8_ps)
            er8 = bpool.tile([NT, 128], F32, tag="er8")
            zq8 = bpool.tile([NT, 1], F32, tag="zq8")
            nc.scalar.activation(out=er8, in_=gt_ps, func=ACTF.Exp, bias=nm8,
                                 accum_out=zq8)
            zq_ps = ps_bk.tile([1, 1], F32, tag="bk")
            nc.tensor.matmul(zq_ps, lhsT=zq8, rhs=ones_col[0:NT, :],
                             start=True, stop=True)
            zqr = bpool.tile([1, 1], F32, tag="zqr")
            nc.vector.reciprocal(out=zqr, in_=zq_ps)
            ec_ps = ps_bk.tile([128, NT], F32, tag="bk")
            nc.tensor.transpose(ec_ps, er8, ident[0:NT, 0:NT])
            ecol = bpool.tile([128, NT], F32R, tag="ecol")
            nc.vector.tensor_copy(out=ecol, in_=ec_ps)
            b1 = ps_work.tile([1, 512], F32, tag="work")
            b2 = ps_work.tile([1, 256], F32, tag="work")
            for t in range(NT):
                nc.tensor.matmul(b1, lhsT=ecol[:, t:t + 1],
                                 rhs=c_big[b][:, t, 0:512], start=(t == 0),
                                 stop=(t == NT - 1))
                nc.tensor.matmul(b2, lhsT=ecol[:, t:t + 1],
                                 rhs=c_big[b][:, t, 512:D], start=(t == 0),
                                 stop=(t == NT - 1))
            bctx = bpool.tile([1, D], F32R, tag="bctx")
            nc.scalar.activation(out=bctx[:, 0:512], in_=b1, func=ACTF.Copy,
                                 scale=zqr)
            nc.scalar.activation(out=bctx[:, 512:D], in_=b2, func=ACTF.Copy,
                                 scale=zqr)
            bb1 = ps_work.tile([128, 512], F32, tag="work")
            bb2 = ps_work.tile([128, 256], F32, tag="work")
            nc.tensor.matmul(bb1, lhsT=ones_row_r, rhs=bctx[:, 0:512],
                             start=True, stop=True)
            nc.tensor.matmul(bb2, lhsT=ones_row_r, rhs=bctx[:, 512:D],
                             start=True, stop=True)
            Bb = qpool.tile([128, D], F32, tag="Bb")
            nc.scalar.copy(out=Bb[:, 0:512], in_=bb1)
            nc.scalar.copy(out=Bb[:, 512:D], in_=bb2)
            for t in range(NT):
                sb = stB.tile([128, D], F32, tag="stB")
                cb_eng = nc.gpsimd if t % 2 == 1 else nc.vector
                cb_eng.tensor_mul(out=sb, in0=_f(c_big[b][:, t, :]), in1=Bb)
                if do_store:
                    nc.sync.dma_start(
                        out=out_d[b, t * 128:(t + 1) * 128, 3 * D:4 * D],
                        in_=sb)
    nc.compile()
    return nc


def _get_program() -> bass.Bass:
    if "nc" not in _CACHE:
        _CACHE["nc"] = _build_program()
    return _CACHE["nc"]


def _make_in_maps(inputs) -> list:
    C = np.ascontiguousarray(np.asarray(inputs["C"], dtype=np.float32))
    Q = np.ascontiguousarray(np.asarray(inputs["Q"], dtype=np.float32))
    c_mask = np.asarray(inputs["c_mask"])
    q_mask = np.asarray(inputs["q_mask"])
    w1 = np.asarray(inputs["w1"], dtype=np.float32).reshape(-1)
    w2 = np.asarray(inputs["w2"], dtype=np.float32).reshape(-1)
    w3 = np.asarray(inputs["w3"], dtype=np.float32).reshape(-1)
    B = C.shape[0]

    qm = q_mask[:, 0, :].astype(np.float32)                     # [B,M] 1=masked
    qbin = 1.0 - qm
    qzb = np.concatenate([Q * qbin[:, :, None], qbin[:, :, None],
                          np.zeros((B, M, 1), np.float32)], axis=2)
    qw3 = np.ascontiguousarray(
        (Q * w3.reshape(1, 1, D)).transpose(0, 2, 1).reshape(B, KC, 128, M))
    qlogit = Q @ w2                                             # [B,M]
    l2 = np.ascontiguousarray(
        np.stack([qlogit, np.ones_like(qlogit)], axis=1))       # [B,2,M]
    clogit = C @ w1                                             # [B,N]
    r2 = np.ascontiguousarray(
        np.stack([np.ones_like(clogit), clogit], axis=1))       # [B,2,N]
    qcorr = np.ascontiguousarray(np.concatenate(
        [np.einsum('bm,bmd->bd', qm, Q), qm.sum(1, keepdims=True),
         np.zeros((B, 1), np.float32)], axis=1).reshape(B, 1, D + 2))
    qmN = np.ascontiguousarray((qm * NEG).reshape(B, 1, M))
    cmf = c_mask[:, 0, :].astype(np.float32)                    # [B,N]
    cmT = np.ascontiguousarray(cmf.reshape(B, NT, 128).transpose(0, 2, 1))
    cmR = np.ascontiguousarray(cmf.reshape(B, 1, N))

    in_maps = []
    for core in range(NCORES):
        sl = slice(core * NB, (core + 1) * NB)
        in_maps.append({
            "C": C[sl],
            "qzb": np.ascontiguousarray(qzb[sl]),
            "qw3": np.ascontiguousarray(qw3[sl]),
            "l2": np.ascontiguousarray(l2[sl]),
            "r2": np.ascontiguousarray(r2[sl]),
            "qcorr": np.ascontiguousarray(qcorr[sl]),
            "cmT": np.ascontiguousarray(cmT[sl]),
            "cmR": np.ascontiguousarray(cmR[sl]),
            "qmN": np.ascontiguousarray(qmN[sl]),
        })
    return in_maps


def kernel(**inputs) -> np.ndarray:
    nc = _get_program()
    in_maps = _make_in_maps(inputs)
    res = run_bass_kernel_spmd(nc, in_maps, list(range(NCORES)))
    return np.concatenate([r["out"] for r in res.results], axis=0)
